# revision 1
# baseline (speedup 1.0000x reference)
"""Trainium2 Bass kernel for nn_ChoiPyramid (Gumbel/Choi pyramid TreeLSTM, eval-mode greedy merge).

Strategy: pure data parallel over batch (16 examples per core, 8 cores).
Per core, dense recompute of all adjacent-pair compositions each level
(matches the reference algorithm), fp32 matmuls (precision required: the
data-dependent argmax merge selection flips with lower-precision matmuls),
merge applied via predicated copies driven by an on-chip row-space argmax.

Layouts (per core, all SBUF tiles partition-major 128):
  state h, c : (128, 4, 16, 48)  = feature-chunk x example x position, fp32
  W^T        : (128, 8, 2560)    = in-feature-chunk x out-feature, fp32
  gates      : psum (128, N) per out-feature chunk, N = examples x pairs
"""
import sys
import os

sys.path.insert(0, "/opt/trn_rl_repo")
import numpy as np

B, L, HID = 128, 48, 512
NCORES = 8
BS = B // NCORES  # 16 examples per core
NEG = -1e30

_built = {}
_last_exec_ns = None


def _build():
    if "nc" in _built:
        return _built
    import concourse.bacc as bacc
    import concourse.mybir as mybir
    from concourse import tile

    F32 = mybir.dt.float32
    U8 = mybir.dt.uint8
    I32 = mybir.dt.int32
    Alu = mybir.AluOpType
    Act = mybir.ActivationFunctionType
    X = mybir.AxisListType.X

    nc = bacc.Bacc("TRN2", target_bir_lowering=False, debug=False, num_devices=NCORES)

    h0_ext = nc.dram_tensor("h0", [128, 4, BS, L], F32, kind="ExternalInput").ap()
    c0_ext = nc.dram_tensor("c0", [128, 4, BS, L], F32, kind="ExternalInput").ap()
    wt_ext = nc.dram_tensor("wt", [128, 8, 5 * HID], F32, kind="ExternalInput").ap()
    badj_ext = nc.dram_tensor("badj", [128, 20], F32, kind="ExternalInput").ap()
    q4_ext = nc.dram_tensor("q4", [128, 4], F32, kind="ExternalInput").ap()
    mbias_ext = nc.dram_tensor("mbias", [1, L - 2, BS, L], F32, kind="ExternalInput").ap()
    kbias_ext = nc.dram_tensor("kbias", [1, L - 1, BS], F32, kind="ExternalInput").ap()
    hout_ext = nc.dram_tensor("hout", [128, 4, BS], F32, kind="ExternalOutput").ap()

    with tile.TileContext(nc) as tc:
        with (
            tc.tile_pool(name="persist", bufs=1) as pp,
            tc.tile_pool(name="work", bufs=1) as wp,
            tc.tile_pool(name="rows", bufs=2) as rp,
            tc.tile_pool(name="rows1", bufs=1) as rp1,
            tc.tile_pool(name="gpsum", bufs=1, space="PSUM") as gp,
            tc.tile_pool(name="lpsum", bufs=2, space="PSUM") as lp,
            tc.tile_pool(name="kpsum", bufs=1, space="PSUM") as kp,
        ):
            # ---------------- persistent tiles ----------------
            wt = pp.tile([128, 8, 5 * HID], F32, tag="wt")
            nc.sync.dma_start(out=wt[:], in_=wt_ext)
            badj = pp.tile([128, 20], F32, tag="badj")
            nc.sync.dma_start(out=badj[:], in_=badj_ext)
            q4 = pp.tile([128, 4], F32, tag="q4")
            nc.sync.dma_start(out=q4[:], in_=q4_ext)
            kbias = pp.tile([1, L - 1, BS], F32, tag="kbias")
            nc.sync.dma_start(out=kbias[:], in_=kbias_ext)

            hbuf = [pp.tile([128, 4, BS, L], F32, tag="hA", name="hA"),
                    pp.tile([128, 4, BS, L], F32, tag="hB", name="hB")]
            cbuf = [pp.tile([128, 4, BS, L], F32, tag="cA", name="cA"),
                    pp.tile([128, 4, BS, L], F32, tag="cB", name="cB")]
            nc.sync.dma_start(out=hbuf[0][:], in_=h0_ext)
            nc.sync.dma_start(out=cbuf[0][:], in_=c0_ext)

            ones = pp.tile([1, 128], F32, tag="ones")
            nc.vector.memset(ones[:], 1.0)
            iorow = pp.tile([1, BS, L], F32, tag="iorow")
            nc.gpsimd.iota(iorow[:], pattern=[[0, BS], [1, L]], base=0,
                           channel_multiplier=0, allow_small_or_imprecise_dtypes=True)
            iof = pp.tile([128, BS, L], F32, tag="iof")
            nc.gpsimd.iota(iof[:], pattern=[[0, BS], [1, L]], base=0,
                           channel_multiplier=0, allow_small_or_imprecise_dtypes=True)
            lrow = pp.tile([1, BS, L], F32, tag="lrow")
            nc.vector.memset(lrow[:], 0.0)

            # ---------------- the 47 levels ----------------
            for i in range(L - 1):
                P = L - 1 - i          # number of adjacent pairs this level
                cur_h, cur_c = hbuf[i % 2], cbuf[i % 2]
                nxt_h, nxt_c = hbuf[(i + 1) % 2], cbuf[(i + 1) % 2]
                nspl = 2 if BS * P > 512 else 1
                bper = BS // nspl

                new_h = wp.tile([128, 4, BS, L - 1], F32, tag="new_h")
                new_c = wp.tile([128, 4, BS, L - 1], F32, tag="new_c")

                for s in range(nspl):
                    b0 = s * bper
                    Rh = bper * P
                    for f in range(4):
                        pg = []
                        for g in range(5):
                            mc = g * 4 + f
                            pt = gp.tile([128, 512], F32, tag=f"g{g}")
                            for kc in range(8):
                                if kc < 4:
                                    rhs = cur_h[:, kc, b0:b0 + bper, 0:P]
                                else:
                                    rhs = cur_h[:, kc - 4, b0:b0 + bper, 1:P + 1]
                                nc.tensor.matmul(
                                    pt[:, 0:Rh].rearrange("p (b j) -> p b j", b=bper),
                                    wt[:, kc, mc * 128:(mc + 1) * 128],
                                    rhs,
                                    start=(kc == 0), stop=(kc == 7),
                                )
                            pg.append(pt)
                        # gates straight out of PSUM (bias folded into ACT)
                        sI = wp.tile([128, 512], F32, tag="sI")
                        sFl = wp.tile([128, 512], F32, tag="sFl")
                        sFr = wp.tile([128, 512], F32, tag="sFr")
                        tU = wp.tile([128, 512], F32, tag="tU")
                        sO = wp.tile([128, 512], F32, tag="sO")
                        nc.scalar.activation(sI[:, 0:Rh], pg[0][:, 0:Rh], Act.Sigmoid,
                                             bias=badj[:, 0 * 4 + f:0 * 4 + f + 1], scale=1.0)
                        nc.scalar.activation(sFl[:, 0:Rh], pg[1][:, 0:Rh], Act.Sigmoid,
                                             bias=badj[:, 1 * 4 + f:1 * 4 + f + 1], scale=1.0)
                        nc.scalar.activation(sFr[:, 0:Rh], pg[2][:, 0:Rh], Act.Sigmoid,
                                             bias=badj[:, 2 * 4 + f:2 * 4 + f + 1], scale=1.0)
                        nc.scalar.activation(tU[:, 0:Rh], pg[3][:, 0:Rh], Act.Tanh,
                                             bias=badj[:, 3 * 4 + f:3 * 4 + f + 1], scale=1.0)
                        nc.scalar.activation(sO[:, 0:Rh], pg[4][:, 0:Rh], Act.Sigmoid,
                                             bias=badj[:, 4 * 4 + f:4 * 4 + f + 1], scale=1.0)
                        cl = cur_c[:, f, b0:b0 + bper, 0:P]
                        cr = cur_c[:, f, b0:b0 + bper, 1:P + 1]
                        t1 = wp.tile([128, 512], F32, tag="t1")
                        t2 = wp.tile([128, 512], F32, tag="t2")
                        t3 = wp.tile([128, 512], F32, tag="t3")
                        t4 = wp.tile([128, 512], F32, tag="t4")
                        nc.vector.tensor_tensor(t1[:, 0:Rh], cl, sFl[:, 0:Rh], op=Alu.mult)
                        nc.vector.tensor_tensor(t2[:, 0:Rh], cr, sFr[:, 0:Rh], op=Alu.mult)
                        nc.vector.tensor_tensor(t3[:, 0:Rh], tU[:, 0:Rh], sI[:, 0:Rh], op=Alu.mult)
                        nc.vector.tensor_tensor(t4[:, 0:Rh], t1[:, 0:Rh], t2[:, 0:Rh], op=Alu.add)
                        ncr = new_c[:, f, b0:b0 + bper, 0:P]
                        nhr = new_h[:, f, b0:b0 + bper, 0:P]
                        nc.vector.tensor_tensor(ncr, t4[:, 0:Rh], t3[:, 0:Rh], op=Alu.add)
                        tch = wp.tile([128, 512], F32, tag="tch")
                        nc.scalar.activation(tch[:, 0:Rh], ncr, Act.Tanh)
                        nc.vector.tensor_tensor(nhr, sO[:, 0:Rh], tch[:, 0:Rh], op=Alu.mult)
                    if i < L - 2:
                        lps = lp.tile([1, 512], F32, tag="lps")
                        for kc in range(4):
                            nc.tensor.matmul(
                                lps[:, 0:Rh].rearrange("p (b j) -> p b j", b=bper),
                                q4[:, kc:kc + 1],
                                new_h[:, kc, b0:b0 + bper, 0:P],
                                start=(kc == 0), stop=(kc == 3),
                            )
                        nc.vector.tensor_copy(
                            lrow[:, b0:b0 + bper, 0:P],
                            lps[:, 0:Rh].rearrange("p (b j) -> p b j", b=bper))

                # ----- merge-selection scores -----
                kst2 = rp1.tile([1, BS], F32, tag="kst2")
                if i < L - 2:
                    mbt = rp.tile([1, BS, L], F32, tag="mbt")
                    nc.sync.dma_start(out=mbt[:], in_=mbias_ext[:, i])
                    msk = rp1.tile([1, BS, L], F32, tag="msk")
                    nc.vector.tensor_tensor(msk[:], lrow[:], mbt[:], op=Alu.add)
                    rmax = rp1.tile([1, BS], F32, tag="rmax")
                    nc.vector.tensor_reduce(rmax[:].unsqueeze(2), msk[:], axis=X, op=Alu.max)
                    eq = rp1.tile([1, BS, L], U8, tag="eq")
                    nc.vector.tensor_tensor(eq[:], msk[:],
                                            rmax[:].unsqueeze(2).broadcast_to([1, BS, L]),
                                            op=Alu.is_ge)
                    cand = rp1.tile([1, BS, L], F32, tag="cand")
                    nc.vector.memset(cand[:], 1e9)
                    nc.vector.copy_predicated(cand[:], eq[:], iorow[:])
                    kst = rp1.tile([1, BS], F32, tag="kst")
                    nc.vector.tensor_reduce(kst[:].unsqueeze(2), cand[:], axis=X, op=Alu.min)
                    nc.vector.tensor_tensor(kst2[:], kst[:], kbias[:, i], op=Alu.add)
                else:
                    nc.vector.tensor_copy(kst2[:], kbias[:, i])

                kcol = kp.tile([128, BS], F32, tag="kcol")
                nc.tensor.matmul(kcol[:], ones[:], kst2[:], start=True, stop=True)
                meq = rp1.tile([128, BS, L], U8, tag="meq")
                mgt = rp1.tile([128, BS, L], U8, tag="mgt")
                kcb = kcol[:, :].unsqueeze(2).broadcast_to([128, BS, L])
                nc.vector.tensor_tensor(meq[:], iof[:], kcb, op=Alu.is_equal)
                nc.vector.tensor_tensor(mgt[:], iof[:], kcb, op=Alu.is_gt)

                # ----- apply merge, per feature chunk (enables overlap) -----
                mgt_b = mgt[:, :, 0:P].unsqueeze(1).broadcast_to([128, 1, BS, P])
                meq_b = meq[:, :, 0:P].unsqueeze(1).broadcast_to([128, 1, BS, P])
                for (nxt, cur, new) in ((nxt_h, cur_h, new_h), (nxt_c, cur_c, new_c)):
                    for f in range(4):
                        dst = nxt[:, f:f + 1, :, 0:P]
                        nc.vector.tensor_copy(dst, cur[:, f:f + 1, :, 0:P])
                        nc.vector.copy_predicated(dst, mgt_b, cur[:, f:f + 1, :, 1:P + 1])
                        nc.vector.copy_predicated(dst, meq_b, new[:, f:f + 1, :, 0:P])

            fin_h = hbuf[(L - 1) % 2]
            nc.sync.dma_start(out=hout_ext, in_=fin_h[:, :, :, 0])

    nc.compile()
    _built["nc"] = nc
    return _built


def _prep_core_inputs(inp_s, length_s, WT128, badj128, q128):
    """Host-side layout prep for one core's shard (BS examples)."""
    h = inp_s[..., :HID]
    c = inp_s[..., HID:]

    def feat_major(x):  # (BS, L, 512) -> (128, 4, BS, L)
        a = np.ascontiguousarray(x.transpose(2, 0, 1))        # (512, BS, L)
        a = a.reshape(4, 128, BS, L).transpose(1, 0, 2, 3)     # (128, 4, BS, L)
        return np.ascontiguousarray(a, dtype=np.float32)

    mbias = np.full((1, L - 2, BS, L), NEG, dtype=np.float32)
    for i in range(L - 2):
        Pn = L - 1 - i
        k = np.arange(Pn)
        valid = (i + 1 + k)[None, :] < length_s[:, None]
        mbias[0, i, :, :Pn] = np.where(valid, 0.0, NEG).astype(np.float32)
    kbias = np.zeros((1, L - 1, BS), dtype=np.float32)
    for i in range(L - 1):
        kbias[0, i, :] = np.where(i + 1 < length_s, 0.0, 1000.0)

    return {
        "h0": feat_major(h),
        "c0": feat_major(c),
        "wt": WT128,
        "badj": badj128,
        "q4": q128,
        "mbias": mbias,
        "kbias": kbias,
    }


def kernel(input, W, b, q, length):
    from concourse.bass_utils import run_bass_kernel_spmd

    built = _build()
    nc = built["nc"]

    input = np.asarray(input, dtype=np.float32)
    W = np.asarray(W, dtype=np.float32)
    b = np.asarray(b, dtype=np.float32)
    q = np.asarray(q, dtype=np.float32)
    length = np.asarray(length).astype(np.int64)

    WT128 = np.ascontiguousarray(
        W.T.reshape(8, 128, 5 * HID).transpose(1, 0, 2), dtype=np.float32)
    badj = b.copy()
    badj[HID:3 * HID] += 1.0  # fl, fr gates get +1.0 folded into bias
    badj128 = np.ascontiguousarray(badj.reshape(20, 128).T, dtype=np.float32)
    q128 = np.ascontiguousarray(q.reshape(4, 128).T, dtype=np.float32)

    in_maps = []
    for cid in range(NCORES):
        sl = slice(cid * BS, (cid + 1) * BS)
        in_maps.append(_prep_core_inputs(input[sl], length[sl], WT128, badj128, q128))

    res = run_bass_kernel_spmd(nc, in_maps, list(range(NCORES)))
    global _last_exec_ns
    _last_exec_ns = getattr(res, "exec_time_ns", None)

    out = np.empty((B, HID), dtype=np.float32)
    for cid in range(NCORES):
        hout = res.results[cid]["hout"]            # (128, 4, BS)
        out[cid * BS:(cid + 1) * BS] = hout.transpose(2, 1, 0).reshape(BS, HID)
    return out


if __name__ == "__main__":
    rng = np.random.default_rng(0)
    inp = {
        "input": rng.standard_normal((B, L, 2 * HID), dtype=np.float32),
        "W": (rng.standard_normal((5 * HID, 2 * HID), dtype=np.float32)
              / np.sqrt(2 * HID)).astype(np.float32),
        "b": np.zeros((5 * HID,), dtype=np.float32),
        "q": (rng.standard_normal((HID,), dtype=np.float32) / np.sqrt(HID)).astype(np.float32),
        "length": rng.integers(L // 2, L + 1, (B,)),
    }
    out = kernel(**inp)
    print("kernel ran, out:", out.shape, out[:2, :4])

